# revision 5
# baseline (speedup 1.0000x reference)
"""Trainium2 Bass kernel for the fused soft-logic-gate layer.

Reference computation:
    pa = softmax(wa, axis=1); pb = softmax(wb, axis=1); pt = softmax(wt, axis=0)
    A = pa @ x; B = pb @ x
    out = sum_g pt[g,:,None] * gate_g(A, B)        (16 soft logic gates)

Every gate is affine in {1, A, B, A*B}, so the 16-gate table collapses to
    out = c0 + cA*A + cB*B + cAB*(A*B)
with four per-row coefficient vectors derived from pt; factoring
    out = (A + u) * (cAB*B + cA) + w,   u = cB/cAB,  w = c0 - cA*u
leaves three elementwise ops per tile, load-balanced across the ACT, DVE and
GpSimd engines.  The matmuls run in fp8e4 DoubleRow perf mode (contraction
256 = 2x128 packed into one instruction at 0.5 cycles/row); the softmax
denominators of wa/wb and the pt normalizer are folded into the epilogue
coefficients, so the raw exp() weights feed the PE directly.  x streams in as
fp8, the output streams out as bf16 (abs tolerance comfortably covers both).

Sharding: batch axis of x split evenly across 8 NeuronCores (data parallel),
weights replicated.
"""

import os
import sys

for _p in ("/opt/trn_rl_repo",):
    if _p not in sys.path and os.path.isdir(_p):
        sys.path.insert(0, _p)

import math

import ml_dtypes
import numpy as np

SIZE = 256
PREV = 256
BATCH = 32768
N_CORES = 8
BSH = BATCH // N_CORES  # per-core batch shard
CH = 1024               # chunk width (A/B PSUM tiles are 2 banks each)
NCH = BSH // CH
P = 128

# wt2 blob layout (f32, [128, 1285]):
#   [:, 0:512]      exp-input for wa, natural layout [p, m, c]
#   [:, 512:1024]   exp-input for wb, natural layout
#   [:16, 1024:1029] sign matrix [16, 5] (cols: sum, c0, cA, cB, cAB)
#   [:16, 1029:1285] wt [16, 256]
WT2_W = 1285

_CACHE = {}

FP8 = ml_dtypes.float8_e4m3
BF16 = ml_dtypes.bfloat16


def _sign_matrix() -> np.ndarray:
    """[16,5] f32 columns: [colsum, c0, cA, cB, cAB] — gate-table
    coefficients of {1, A, B, A*B} preceded by the softmax denominator."""
    S = np.zeros((16, 5), dtype=np.float32)
    S[:, 0] = 1.0
    S[8:16, 1] = 1.0
    for g in (2, 3, 6, 7):
        S[g, 2] += 1.0
    for g in (8, 9, 12, 13):
        S[g, 2] -= 1.0
    for g in (4, 5, 6, 7):
        S[g, 3] += 1.0
    for g in (8, 9, 10, 11):
        S[g, 3] -= 1.0
    for g, v in {1: 1, 2: -1, 4: -1, 6: -2, 7: -1, 8: 1, 9: 2, 11: 1, 13: 1, 14: -1}.items():
        S[g, 4] = v
    return S


def _build_bass():
    import concourse.bacc as bacc
    import concourse.tile as tile
    import concourse.mybir as mybir

    f32 = mybir.dt.float32
    f8 = mybir.dt.float8e4
    bf = mybir.dt.bfloat16
    Act = mybir.ActivationFunctionType
    Alu = mybir.AluOpType
    DR = mybir.MatmulPerfMode.DoubleRow

    nc = bacc.Bacc(trn_type="TRN2", target_bir_lowering=False, debug=False,
                   num_devices=N_CORES)

    # transposed, shifted exp-inputs: [p, which(a/b), k-block, out-row]
    wt1_d = nc.dram_tensor("wt1", [P, 2, 2, SIZE], f32, kind="ExternalInput").ap()
    wt2_d = nc.dram_tensor("wt2", [P, WT2_W], f32, kind="ExternalInput").ap()
    xs_d = nc.dram_tensor("xs", [P, 2, BSH], f8, kind="ExternalInput").ap()
    out_d = nc.dram_tensor("out", [P, 2, BSH], f32, kind="ExternalOutput").ap()

    # epilogue engine schedule per (n, m) tile.  GPSIMD cannot read PSUM, so
    # op2 (reads A psum) is DVE-only and op1 (reads B psum) is ACT; the
    # SBUF-only op3 goes mostly to GpSimd with ACT/DVE absorbing the rest.
    # V=vector(DVE), A=scalar(ACT), G=gpsimd(Pool).
    OP2 = {(n, m): 'V' for n in range(NCH) for m in range(2)}
    OP3 = {(0, 0): 'G', (0, 1): 'G', (1, 0): 'G', (1, 1): 'A',
           (2, 0): 'G', (2, 1): 'G', (3, 0): 'V', (3, 1): 'G'}

    with tile.TileContext(nc) as tc:
        with tc.tile_pool(name="consts", bufs=1) as consts, \
             tc.tile_pool(name="weights", bufs=1) as weights, \
             tc.tile_pool(name="coefs", bufs=1) as coefs, \
             tc.tile_pool(name="xp", bufs=NCH) as xp:

            # tiny early Exp forces the ACT table load off the critical path
            seed = consts.tile([1, 1], f32)
            nc.vector.memset(seed[:], 0.0)
            dummy = consts.tile([1, 1], f32)
            nc.scalar.activation(out=dummy[:], in_=seed[:], func=Act.Exp)

            # weights: DMA transposed exp-inputs (a first — A matmuls go first)
            wt1 = consts.tile([P, 2, 2, SIZE], f32)
            nc.sync.dma_start(out=wt1[:, 0], in_=wt1_d[:, 0])
            nc.sync.dma_start(out=wt1[:, 1], in_=wt1_d[:, 1])
            wt2 = consts.tile([P, WT2_W], f32)
            nc.sync.dma_start(out=wt2[:], in_=wt2_d[:])

            # all x chunks up front (8KB/partition total in fp8)
            xtiles = []
            for n in range(NCH):
                xt = xp.tile([P, 2, CH], f8, tag="x", name=f"x{n}")
                nc.sync.dma_start(out=xt[:], in_=xs_d[:, :, n * CH:(n + 1) * CH])
                xtiles.append(xt)

            # exp -> fp8 DoubleRow weights, one ACT op per matrix
            e8T = weights.tile([P, 2, 2, SIZE], f8, tag="e8T")
            for wsel in range(2):
                nc.scalar.activation(out=e8T[:, wsel], in_=wt1[:, wsel], func=Act.Exp)

            # natural-layout exp (f32) for the row sums
            e_nat = weights.tile([P, 2, 2, SIZE], f32, tag="e_nat")
            nc.scalar.activation(out=e_nat[:],
                                 in_=wt2[:, 0:1024].rearrange("p (w m c) -> p w m c", w=2, m=2),
                                 func=Act.Exp)

            # [128,2] coefficient tiles (m as free dim):
            sc2 = coefs.tile([P, 2], f32, tag="sc2")   # op1 scale
            bi2 = coefs.tile([P, 2], f32, tag="bi2")   # op1 bias
            U2 = coefs.tile([P, 2], f32, tag="U2")     # op2 scalar
            w2 = coefs.tile([P, 2], f32, tag="w2")     # op3 scalar

            # ---- coefficient preprocessing ----
            with tc.tile_pool(name="prep", bufs=1) as prep, \
                 tc.tile_pool(name="coef_ps", bufs=1, space="PSUM") as coef_ps:

                smat = wt2[:16, 1024:1029]
                wts = wt2[:16, 1029:1285]

                # row sums of exp(w - shift): [p, which, m]
                rs4 = prep.tile([P, 2, 2], f32, tag="rs4")
                for wsel in range(2):
                    for m in range(2):
                        nc.vector.tensor_reduce(out=rs4[:, wsel, m:m + 1],
                                                in_=e_nat[:, wsel, m, :],
                                                axis=mybir.AxisListType.X, op=Alu.add)

                # pt-coefficient path
                ept = prep.tile([16, SIZE], f32, tag="ept")
                nc.scalar.activation(out=ept[:], in_=wts, func=Act.Exp)
                cps = coef_ps.tile([P, 10], f32, tag="cps")
                for m in range(2):
                    nc.tensor.matmul(cps[:, m * 5:(m + 1) * 5],
                                     ept[:, m * P:(m + 1) * P], smat,
                                     start=True, stop=True)
                cpsv = cps[:].rearrange("p (m c) -> p c m", m=2)

                sa2 = rs4[:, 0, :]
                sb2 = rs4[:, 1, :]
                rpts = prep.tile([P, 2], f32, tag="rpts")
                nc.vector.reciprocal(out=rpts[:], in_=cpsv[:, 0, :])
                rcab = prep.tile([P, 2], f32, tag="rcab")
                nc.vector.reciprocal(out=rcab[:], in_=cpsv[:, 4, :])
                rsa = prep.tile([P, 2], f32, tag="rsa")
                nc.vector.reciprocal(out=rsa[:], in_=sa2)
                rsb = prep.tile([P, 2], f32, tag="rsb")
                nc.vector.reciprocal(out=rsb[:], in_=sb2)

                h = prep.tile([P, 2], f32, tag="h")
                nc.vector.tensor_tensor(out=h[:], in0=rpts[:], in1=rsa[:], op=Alu.mult)
                nc.vector.tensor_tensor(out=bi2[:], in0=cpsv[:, 2, :], in1=h[:], op=Alu.mult)
                h2 = prep.tile([P, 2], f32, tag="h2")
                nc.vector.tensor_tensor(out=h2[:], in0=h[:], in1=rsb[:], op=Alu.mult)
                nc.vector.tensor_tensor(out=sc2[:], in0=cpsv[:, 4, :], in1=h2[:], op=Alu.mult)
                g = prep.tile([P, 2], f32, tag="g")
                nc.vector.tensor_tensor(out=g[:], in0=cpsv[:, 3, :], in1=rcab[:], op=Alu.mult)
                nc.vector.tensor_tensor(out=U2[:], in0=g[:], in1=sa2, op=Alu.mult)
                t = prep.tile([P, 2], f32, tag="t")
                nc.vector.tensor_tensor(out=t[:], in0=cpsv[:, 2, :], in1=g[:], op=Alu.mult)
                t2 = prep.tile([P, 2], f32, tag="t2")
                nc.vector.tensor_tensor(out=t2[:], in0=cpsv[:, 1, :], in1=t[:], op=Alu.subtract)
                nc.vector.tensor_tensor(out=w2[:], in0=t2[:], in1=rpts[:], op=Alu.mult)

            # ---- main loop ----
            eng = {'V': nc.vector, 'A': nc.scalar, 'G': nc.gpsimd}
            with tc.tile_pool(name="ep", bufs=2) as ep, \
                 tc.tile_pool(name="mm_ps", bufs=1, space="PSUM") as mm_ps:
                for n in range(NCH):
                    xk = xtiles[n]
                    o_sb = ep.tile([P, 2, CH], f32, tag="o", name=f"o{n}")
                    for m in range(2):
                        a_ps = mm_ps.tile([P, CH], f32, tag=f"A{m}", name=f"A{n}{m}")
                        b_ps = mm_ps.tile([P, CH], f32, tag=f"B{m}", name=f"B{n}{m}")
                        for ps_t, wsel in ((a_ps, 0), (b_ps, 1)):
                            wT = e8T[:, wsel, :, m * P:(m + 1) * P]
                            for s in range(CH // 512):
                                sl = slice(s * 512, (s + 1) * 512)
                                nc.tensor.matmul(ps_t[:, sl], wT, xk[:, :, sl],
                                                 start=True, stop=True, perf_mode=DR)
                        # out = (A + U) * (sc*B + bi) + w
                        s_sb = ep.tile([P, CH], f32, tag="s", name=f"s{n}{m}")
                        nc.scalar.activation(out=s_sb[:], in_=b_ps[:], func=Act.Identity,
                                             scale=sc2[:, m:m + 1], bias=bi2[:, m:m + 1])
                        p_sb = ep.tile([P, CH], f32, tag="p", name=f"p{n}{m}")
                        eng[OP2[(n, m)]].scalar_tensor_tensor(
                            out=p_sb[:], in0=a_ps[:], scalar=U2[:, m:m + 1],
                            in1=s_sb[:], op0=Alu.add, op1=Alu.mult)
                        if OP3[(n, m)] == 'A':
                            nc.scalar.activation(out=o_sb[:, m, :], in_=p_sb[:],
                                                 func=Act.Identity, bias=w2[:, m:m + 1])
                        else:
                            eng[OP3[(n, m)]].tensor_scalar_add(
                                o_sb[:, m, :], p_sb[:], w2[:, m:m + 1])
                    nc.sync.dma_start(out=out_d[:, :, n * CH:(n + 1) * CH], in_=o_sb[:])

    nc.compile()
    return nc


def _get_nc():
    if "nc" not in _CACHE:
        _CACHE["nc"] = _build_bass()
    return _CACHE["nc"]


def _prep_weights(wa, wb, wt):
    """Host-side layout/dtype prep: stability shift + transpose of the exp
    inputs (softmax itself — exp, sums, normalization — runs on device)."""
    lg = math.log(128.0)
    sha = wa.max(axis=1, keepdims=True) - lg
    shb = wb.max(axis=1, keepdims=True) - lg

    def tr(w, sh):  # [size, prev] -> [p, kblock, size]
        return np.ascontiguousarray(
            (w - sh).T.reshape(2, P, SIZE).transpose(1, 0, 2))

    def nat(w, sh):  # [size, prev] -> [p, (m, c)]
        return (w - sh).reshape(2, P, PREV).transpose(1, 0, 2).reshape(P, 2 * PREV)

    wt1 = np.stack([tr(wa, sha), tr(wb, shb)], axis=1)  # [p, which, k, size]

    wt2 = np.zeros((P, WT2_W), dtype=np.float32)
    wt2[:, 0:512] = nat(wa, sha)
    wt2[:, 512:1024] = nat(wb, shb)
    wt2[:16, 1024:1029] = _sign_matrix()
    wt2[:16, 1029:1285] = wt
    return np.ascontiguousarray(wt1), wt2


def _run(x, wa, wb, wt, trace=False, **spmd_kwargs):
    from concourse import bass_utils

    nc = _get_nc()
    x = np.asarray(x, dtype=np.float32)
    wa = np.asarray(wa, dtype=np.float32)
    wb = np.asarray(wb, dtype=np.float32)
    wt = np.asarray(wt, dtype=np.float32)
    wt1, wt2 = _prep_weights(wa, wb, wt)

    # [256, batch] f32 -> [p, kblock, batch] fp8
    x8 = np.ascontiguousarray(
        x.astype(FP8).reshape(2, P, BATCH).transpose(1, 0, 2))

    in_maps = []
    for c in range(N_CORES):
        in_maps.append({
            "xs": np.ascontiguousarray(x8[:, :, c * BSH:(c + 1) * BSH]),
            "wt1": wt1, "wt2": wt2,
        })
    res = bass_utils.run_bass_kernel_spmd(nc, in_maps, core_ids=list(range(N_CORES)),
                                          trace=trace, **spmd_kwargs)
    outs = []
    for c in range(N_CORES):
        o = np.asarray(res.results[c]["out"])          # [p, m, bsh] bf16
        outs.append(o.astype(np.float32).transpose(1, 0, 2).reshape(SIZE, BSH))
    return np.concatenate(outs, axis=1), res


def kernel(x, wa, wb, wt):
    out, _ = _run(x, wa, wb, wt, trace=False)
    return out


# revision 8
# speedup vs baseline: 1.9104x; 1.9104x over previous
"""Trainium2 Bass kernel for the fused soft-logic-gate layer.

Reference computation:
    pa = softmax(wa, axis=1); pb = softmax(wb, axis=1); pt = softmax(wt, axis=0)
    A = pa @ x; B = pb @ x
    out = sum_g pt[g,:,None] * gate_g(A, B)        (16 soft logic gates)

Every gate is affine in {1, A, B, A*B}, so the 16-gate table collapses to
    out = c0 + cA*A + cB*B + cAB*(A*B)
with four per-row coefficient vectors derived from pt; factoring
    out = (A + u) * (cAB*B + cA) + w,   u = cB/cAB,  w = c0 - cA*u
leaves three elementwise ops per tile, load-balanced across the ACT, DVE and
GpSimd engines.  The matmuls run in fp8e4 DoubleRow perf mode (contraction
256 = 2x128 packed into one instruction at 0.5 cycles/row); the softmax
denominators of wa/wb and the pt normalizer are folded into the epilogue
coefficients, so the raw exp() weights feed the PE directly.  x streams in as
fp8, the output streams out as bf16 (abs tolerance comfortably covers both).

Sharding: batch axis of x split evenly across 8 NeuronCores (data parallel),
weights replicated.
"""

import os
import sys

for _p in ("/opt/trn_rl_repo",):
    if _p not in sys.path and os.path.isdir(_p):
        sys.path.insert(0, _p)

import math

import ml_dtypes
import numpy as np

SIZE = 256
PREV = 256
BATCH = 32768
N_CORES = 8
BSH = BATCH // N_CORES  # per-core batch shard
CH = 1024               # chunk width (A/B PSUM tiles are 2 banks each)
NCH = BSH // CH
P = 128

# wt2 blob layout (f32, [128, 1285]):
#   [:, 0:512]      exp-input for wa, natural layout [p, m, c]
#   [:, 512:1024]   exp-input for wb, natural layout
#   [:16, 1024:1029] sign matrix [16, 5] (cols: sum, c0, cA, cB, cAB)
#   [:16, 1029:1285] wt [16, 256]
WT2_W = 1285

_CACHE = {}

FP8 = ml_dtypes.float8_e4m3
BF16 = ml_dtypes.bfloat16


def _sign_matrix() -> np.ndarray:
    """[16,5] f32 columns: [colsum, c0, cA, cB, cAB] — gate-table
    coefficients of {1, A, B, A*B} preceded by the softmax denominator."""
    S = np.zeros((16, 5), dtype=np.float32)
    S[:, 0] = 1.0
    S[8:16, 1] = 1.0
    for g in (2, 3, 6, 7):
        S[g, 2] += 1.0
    for g in (8, 9, 12, 13):
        S[g, 2] -= 1.0
    for g in (4, 5, 6, 7):
        S[g, 3] += 1.0
    for g in (8, 9, 10, 11):
        S[g, 3] -= 1.0
    for g, v in {1: 1, 2: -1, 4: -1, 6: -2, 7: -1, 8: 1, 9: 2, 11: 1, 13: 1, 14: -1}.items():
        S[g, 4] = v
    return S


def _build_bass():
    import concourse.bacc as bacc
    import concourse.tile as tile
    import concourse.mybir as mybir

    f32 = mybir.dt.float32
    f8 = mybir.dt.float8e4
    bf = mybir.dt.bfloat16
    Act = mybir.ActivationFunctionType
    Alu = mybir.AluOpType
    DR = mybir.MatmulPerfMode.DoubleRow

    nc = bacc.Bacc(trn_type="TRN2", target_bir_lowering=False, debug=False,
                   num_devices=N_CORES)

    # transposed, shifted exp-inputs: [p, which(a/b), k-block, out-row]
    wt1_d = nc.dram_tensor("wt1", [P, 2, 2, SIZE], f32, kind="ExternalInput").ap()
    wt2_d = nc.dram_tensor("wt2", [P, WT2_W], f32, kind="ExternalInput").ap()
    xs_d = nc.dram_tensor("xs", [P, 2, BSH], f8, kind="ExternalInput").ap()
    out_d = nc.dram_tensor("out", [P, 2, BSH], f32, kind="ExternalOutput").ap()

    # Epilogue split: GPSIMD cannot read PSUM and its elementwise ucode is
    # ~14 ns/elem (useless), so op1 (affine of B psum) runs on ACT, op2
    # ((A+U)*s) on DVE, and the +w pass rides a GpSimd-issued accumulate-DMA
    # (cce_op=add): the DMA fabric adds a broadcast w image into the p tile,
    # costing no ACT/DVE cycles.

    with tile.TileContext(nc) as tc:
        with tc.tile_pool(name="consts", bufs=1) as consts, \
             tc.tile_pool(name="weights", bufs=1) as weights, \
             tc.tile_pool(name="coefs", bufs=1) as coefs, \
             tc.tile_pool(name="xp", bufs=NCH) as xp:

            # tiny early Exp forces the ACT table load off the critical path
            seed = consts.tile([1, 1], f32)
            nc.vector.memset(seed[:], 0.0)
            dummy = consts.tile([1, 1], f32)
            nc.scalar.activation(out=dummy[:], in_=seed[:], func=Act.Exp)

            # weights: DMA transposed exp-inputs (a first — A matmuls go first)
            wt1 = consts.tile([P, 2, 2, SIZE], f32)
            nc.sync.dma_start(out=wt1[:, 0], in_=wt1_d[:, 0])
            nc.sync.dma_start(out=wt1[:, 1], in_=wt1_d[:, 1])
            wt2 = consts.tile([P, WT2_W], f32)
            nc.sync.dma_start(out=wt2[:], in_=wt2_d[:])

            # all x chunks up front (8KB/partition total in fp8)
            xtiles = []
            for n in range(NCH):
                xt = xp.tile([P, 2, CH], f8, tag="x", name=f"x{n}")
                nc.sync.dma_start(out=xt[:], in_=xs_d[:, :, n * CH:(n + 1) * CH])
                xtiles.append(xt)

            # exp -> fp8 DoubleRow weights, one ACT op per matrix
            e8T = weights.tile([P, 2, 2, SIZE], f8, tag="e8T")
            for wsel in range(2):
                nc.scalar.activation(out=e8T[:, wsel], in_=wt1[:, wsel], func=Act.Exp)

            # natural-layout exp (f32) for the row sums
            e_nat = weights.tile([P, 2, 2, SIZE], f32, tag="e_nat")
            nc.scalar.activation(out=e_nat[:],
                                 in_=wt2[:, 0:1024].rearrange("p (w m c) -> p w m c", w=2, m=2),
                                 func=Act.Exp)

            # [128,2] coefficient tiles (m as free dim):
            sc2 = coefs.tile([P, 2], f32, tag="sc2")   # op1 scale
            bi2 = coefs.tile([P, 2], f32, tag="bi2")   # op1 bias
            U2 = coefs.tile([P, 2], f32, tag="U2")     # op2 scalar
            w2 = coefs.tile([P, 2], f32, tag="w2")     # op3 scalar

            # ---- coefficient preprocessing ----
            with tc.tile_pool(name="prep", bufs=1) as prep, \
                 tc.tile_pool(name="coef_ps", bufs=1, space="PSUM") as coef_ps:

                smat = wt2[:16, 1024:1029]
                wts = wt2[:16, 1029:1285]

                # row sums of exp(w - shift): [p, which, m]
                rs4 = prep.tile([P, 2, 2], f32, tag="rs4")
                for wsel in range(2):
                    for m in range(2):
                        nc.vector.tensor_reduce(out=rs4[:, wsel, m:m + 1],
                                                in_=e_nat[:, wsel, m, :],
                                                axis=mybir.AxisListType.X, op=Alu.add)

                # pt-coefficient path
                ept = prep.tile([16, SIZE], f32, tag="ept")
                nc.scalar.activation(out=ept[:], in_=wts, func=Act.Exp)
                cps = coef_ps.tile([P, 10], f32, tag="cps")
                for m in range(2):
                    nc.tensor.matmul(cps[:, m * 5:(m + 1) * 5],
                                     ept[:, m * P:(m + 1) * P], smat,
                                     start=True, stop=True)
                cpsv = cps[:].rearrange("p (m c) -> p c m", m=2)

                sa2 = rs4[:, 0, :]
                sb2 = rs4[:, 1, :]
                rpts = prep.tile([P, 2], f32, tag="rpts")
                nc.vector.reciprocal(out=rpts[:], in_=cpsv[:, 0, :])
                rcab = prep.tile([P, 2], f32, tag="rcab")
                nc.vector.reciprocal(out=rcab[:], in_=cpsv[:, 4, :])
                rsa = prep.tile([P, 2], f32, tag="rsa")
                nc.vector.reciprocal(out=rsa[:], in_=sa2)
                rsb = prep.tile([P, 2], f32, tag="rsb")
                nc.vector.reciprocal(out=rsb[:], in_=sb2)

                h = prep.tile([P, 2], f32, tag="h")
                nc.vector.tensor_tensor(out=h[:], in0=rpts[:], in1=rsa[:], op=Alu.mult)
                nc.vector.tensor_tensor(out=bi2[:], in0=cpsv[:, 2, :], in1=h[:], op=Alu.mult)
                h2 = prep.tile([P, 2], f32, tag="h2")
                nc.vector.tensor_tensor(out=h2[:], in0=h[:], in1=rsb[:], op=Alu.mult)
                nc.vector.tensor_tensor(out=sc2[:], in0=cpsv[:, 4, :], in1=h2[:], op=Alu.mult)
                g = prep.tile([P, 2], f32, tag="g")
                nc.vector.tensor_tensor(out=g[:], in0=cpsv[:, 3, :], in1=rcab[:], op=Alu.mult)
                nc.vector.tensor_tensor(out=U2[:], in0=g[:], in1=sa2, op=Alu.mult)
                t = prep.tile([P, 2], f32, tag="t")
                nc.vector.tensor_tensor(out=t[:], in0=cpsv[:, 2, :], in1=g[:], op=Alu.mult)
                t2 = prep.tile([P, 2], f32, tag="t2")
                nc.vector.tensor_tensor(out=t2[:], in0=cpsv[:, 1, :], in1=t[:], op=Alu.subtract)
                nc.vector.tensor_tensor(out=w2[:], in0=t2[:], in1=rpts[:], op=Alu.mult)

            # broadcast w image for the accumulate-DMA (+w pass)
            w_img = coefs.tile([P, 2, CH], f32, tag="w_img")
            junk = wt2[:, 0:CH]
            for m in range(2):
                nc.vector.tensor_scalar(w_img[:, m, :], junk, 0.0, w2[:, m:m + 1],
                                        Alu.mult, Alu.add)

            # ---- main loop ----
            with tc.tile_pool(name="ep", bufs=2) as ep, \
                 tc.tile_pool(name="mm_ps", bufs=1, space="PSUM") as mm_ps:
                for n in range(NCH):
                    xk = xtiles[n]
                    p_sb = ep.tile([P, 2, CH], f32, tag="p", name=f"p{n}")
                    for m in range(2):
                        a_ps = mm_ps.tile([P, CH], f32, tag=f"A{m}", name=f"A{n}{m}")
                        b_ps = mm_ps.tile([P, CH], f32, tag=f"B{m}", name=f"B{n}{m}")
                        for ps_t, wsel in ((a_ps, 0), (b_ps, 1)):
                            wT = e8T[:, wsel, :, m * P:(m + 1) * P]
                            for s in range(CH // 512):
                                sl = slice(s * 512, (s + 1) * 512)
                                nc.tensor.matmul(ps_t[:, sl], wT, xk[:, :, sl],
                                                 start=True, stop=True, perf_mode=DR)
                        # out = (A + U) * (sc*B + bi) + w
                        s_sb = ep.tile([P, CH], f32, tag="s", name=f"s{n}{m}")
                        nc.scalar.activation(out=s_sb[:], in_=b_ps[:], func=Act.Identity,
                                             scale=sc2[:, m:m + 1], bias=bi2[:, m:m + 1])
                        nc.vector.scalar_tensor_tensor(
                            out=p_sb[:, m, :], in0=a_ps[:], scalar=U2[:, m:m + 1],
                            in1=s_sb[:], op0=Alu.add, op1=Alu.mult)
                    nc.gpsimd.dma_start(out=p_sb[:], in_=w_img[:], accum_op=Alu.add)
                    nc.sync.dma_start(out=out_d[:, :, n * CH:(n + 1) * CH], in_=p_sb[:])

    nc.compile()
    return nc


def _get_nc():
    if "nc" not in _CACHE:
        _CACHE["nc"] = _build_bass()
    return _CACHE["nc"]


def _prep_weights(wa, wb, wt):
    """Host-side layout/dtype prep: stability shift + transpose of the exp
    inputs (softmax itself — exp, sums, normalization — runs on device)."""
    lg = math.log(128.0)
    sha = wa.max(axis=1, keepdims=True) - lg
    shb = wb.max(axis=1, keepdims=True) - lg

    def tr(w, sh):  # [size, prev] -> [p, kblock, size]
        return np.ascontiguousarray(
            (w - sh).T.reshape(2, P, SIZE).transpose(1, 0, 2))

    def nat(w, sh):  # [size, prev] -> [p, (m, c)]
        return (w - sh).reshape(2, P, PREV).transpose(1, 0, 2).reshape(P, 2 * PREV)

    wt1 = np.stack([tr(wa, sha), tr(wb, shb)], axis=1)  # [p, which, k, size]

    wt2 = np.zeros((P, WT2_W), dtype=np.float32)
    wt2[:, 0:512] = nat(wa, sha)
    wt2[:, 512:1024] = nat(wb, shb)
    wt2[:16, 1024:1029] = _sign_matrix()
    wt2[:16, 1029:1285] = wt
    return np.ascontiguousarray(wt1), wt2


def _run(x, wa, wb, wt, trace=False, **spmd_kwargs):
    from concourse import bass_utils

    nc = _get_nc()
    x = np.asarray(x, dtype=np.float32)
    wa = np.asarray(wa, dtype=np.float32)
    wb = np.asarray(wb, dtype=np.float32)
    wt = np.asarray(wt, dtype=np.float32)
    wt1, wt2 = _prep_weights(wa, wb, wt)

    # [256, batch] f32 -> [p, kblock, batch] fp8
    x8 = np.ascontiguousarray(
        x.astype(FP8).reshape(2, P, BATCH).transpose(1, 0, 2))

    in_maps = []
    for c in range(N_CORES):
        in_maps.append({
            "xs": np.ascontiguousarray(x8[:, :, c * BSH:(c + 1) * BSH]),
            "wt1": wt1, "wt2": wt2,
        })
    res = bass_utils.run_bass_kernel_spmd(nc, in_maps, core_ids=list(range(N_CORES)),
                                          trace=trace, **spmd_kwargs)
    outs = []
    for c in range(N_CORES):
        o = np.asarray(res.results[c]["out"])          # [p, m, bsh] bf16
        outs.append(o.astype(np.float32).transpose(1, 0, 2).reshape(SIZE, BSH))
    return np.concatenate(outs, axis=1), res


def kernel(x, wa, wb, wt):
    out, _ = _run(x, wa, wb, wt, trace=False)
    return out


# revision 10
# speedup vs baseline: 3.3221x; 1.7389x over previous
"""Trainium2 Bass kernel for the fused soft-logic-gate layer.

Reference computation:
    pa = softmax(wa, axis=1); pb = softmax(wb, axis=1); pt = softmax(wt, axis=0)
    A = pa @ x; B = pb @ x
    out = sum_g pt[g,:,None] * gate_g(A, B)        (16 soft logic gates)

Every gate is affine in {1, A, B, A*B}, so the 16-gate table collapses to
    out = c0 + cA*A + cB*B + cAB*(A*B)
with four per-row coefficient vectors derived from pt; factoring
    out = (A + u) * (cAB*B + cA) + w,   u = cB/cAB,  w = c0 - cA*u
leaves three elementwise ops per tile, load-balanced across the ACT, DVE and
GpSimd engines.  The matmuls run in fp8e4 DoubleRow perf mode (contraction
256 = 2x128 packed into one instruction at 0.5 cycles/row); the softmax
denominators of wa/wb and the pt normalizer are folded into the epilogue
coefficients, so the raw exp() weights feed the PE directly.  x streams in as
fp8, the output streams out as bf16 (abs tolerance comfortably covers both).

Sharding: batch axis of x split evenly across 8 NeuronCores (data parallel),
weights replicated.
"""

import os
import sys

for _p in ("/opt/trn_rl_repo",):
    if _p not in sys.path and os.path.isdir(_p):
        sys.path.insert(0, _p)

import math

import ml_dtypes
import numpy as np

SIZE = 256
PREV = 256
BATCH = 32768
N_CORES = 8
BSH = BATCH // N_CORES  # per-core batch shard
CH = 1024               # chunk width (A/B PSUM tiles are 2 banks each)
NCH = BSH // CH
P = 128

# wt2 blob layout (f32, [128, 1285]):
#   [:, 0:512]      exp-input for wa, natural layout [p, m, c]
#   [:, 512:1024]   exp-input for wb, natural layout
#   [:16, 1024:1029] sign matrix [16, 5] (cols: sum, c0, cA, cB, cAB)
#   [:16, 1029:1285] wt [16, 256]
WT2_W = 1285

_CACHE = {}

FP8 = ml_dtypes.float8_e4m3
BF16 = ml_dtypes.bfloat16


def _sign_matrix() -> np.ndarray:
    """[16,5] f32 columns: [colsum, c0, cA, cB, cAB] — gate-table
    coefficients of {1, A, B, A*B} preceded by the softmax denominator."""
    S = np.zeros((16, 5), dtype=np.float32)
    S[:, 0] = 1.0
    S[8:16, 1] = 1.0
    for g in (2, 3, 6, 7):
        S[g, 2] += 1.0
    for g in (8, 9, 12, 13):
        S[g, 2] -= 1.0
    for g in (4, 5, 6, 7):
        S[g, 3] += 1.0
    for g in (8, 9, 10, 11):
        S[g, 3] -= 1.0
    for g, v in {1: 1, 2: -1, 4: -1, 6: -2, 7: -1, 8: 1, 9: 2, 11: 1, 13: 1, 14: -1}.items():
        S[g, 4] = v
    return S


def _build_bass():
    import concourse.bacc as bacc
    import concourse.tile as tile
    import concourse.mybir as mybir

    f32 = mybir.dt.float32
    f8 = mybir.dt.float8e4
    bf = mybir.dt.bfloat16
    Act = mybir.ActivationFunctionType
    Alu = mybir.AluOpType
    DR = mybir.MatmulPerfMode.DoubleRow

    nc = bacc.Bacc(trn_type="TRN2", target_bir_lowering=False, debug=False,
                   num_devices=N_CORES)

    # transposed, shifted exp-inputs: [p, which(a/b), k-block, out-row]
    wt1_d = nc.dram_tensor("wt1", [P, 2, 2, SIZE], f32, kind="ExternalInput").ap()
    wt2_d = nc.dram_tensor("wt2", [P, WT2_W], f32, kind="ExternalInput").ap()
    xs_d = nc.dram_tensor("xs", [P, 2, BSH], f8, kind="ExternalInput").ap()
    outf_d = nc.dram_tensor("outf", [P, BSH], f32, kind="ExternalOutput").ap()
    outb_d = nc.dram_tensor("outb", [P, BSH], bf, kind="ExternalOutput").ap()

    # Epilogue split: GPSIMD cannot read PSUM and its elementwise ucode is
    # ~14 ns/elem (useless), so everything runs on ACT + DVE: op1 (affine of
    # B psum) on ACT, op2 ((A+U)*s) on DVE, and op3 (+w) alternating between
    # ACT (bf16 out -- its output cast is free) and DVE (f32 out -- its bf16
    # store path is a ~19x-slow scalar fallback).

    with tile.TileContext(nc) as tc:
        with tc.tile_pool(name="consts", bufs=1) as consts, \
             tc.tile_pool(name="weights", bufs=1) as weights, \
             tc.tile_pool(name="coefs", bufs=1) as coefs, \
             tc.tile_pool(name="xp", bufs=NCH) as xp:

            # tiny early Exp forces the ACT table load off the critical path
            seed = consts.tile([1, 1], f32)
            nc.vector.memset(seed[:], 0.0)
            dummy = consts.tile([1, 1], f32)
            nc.scalar.activation(out=dummy[:], in_=seed[:], func=Act.Exp)

            # weight DMAs, deepest-dependency-chain first: misc (pt path),
            # wa-transposed (first matmuls), naturals (row sums), wb-transposed
            wt2 = consts.tile([P, WT2_W], f32)
            nc.sync.dma_start(out=wt2[:16, 1024:], in_=wt2_d[:16, 1024:])
            wt1 = consts.tile([P, 2, 2, SIZE], f32)
            nc.sync.dma_start(out=wt1[:, 0], in_=wt1_d[:, 0])
            nc.sync.dma_start(out=wt2[:, 0:1024], in_=wt2_d[:, 0:1024])
            nc.sync.dma_start(out=wt1[:, 1], in_=wt1_d[:, 1])

            # all x chunks up front (8KB/partition total in fp8)
            xtiles = []
            for n in range(NCH):
                xt = xp.tile([P, 2, CH], f8, tag="x", name=f"x{n}")
                nc.sync.dma_start(out=xt[:], in_=xs_d[:, :, n * CH:(n + 1) * CH])
                xtiles.append(xt)

            e8T = weights.tile([P, 2, 2, SIZE], f8, tag="e8T")
            e_nat = weights.tile([P, 2, 2, SIZE], f32, tag="e_nat")

            # [128,2] coefficient tiles (m as free dim):
            sc2 = coefs.tile([P, 2], f32, tag="sc2")   # op1 scale
            bi2 = coefs.tile([P, 2], f32, tag="bi2")   # op1 bias
            U2 = coefs.tile([P, 2], f32, tag="U2")     # op2 scalar
            w2 = coefs.tile([P, 2], f32, tag="w2")     # op3 scalar

            # ---- coefficient preprocessing ----
            with tc.tile_pool(name="prep", bufs=1) as prep, \
                 tc.tile_pool(name="coef_ps", bufs=1, space="PSUM") as coef_ps:

                smat = wt2[:16, 1024:1029]
                wts = wt2[:16, 1029:1285]

                # pt-coefficient path first: it needs only the tiny misc DMA
                ept = prep.tile([16, SIZE], f32, tag="ept")
                nc.scalar.activation(out=ept[:], in_=wts, func=Act.Exp)
                cps = coef_ps.tile([P, 10], f32, tag="cps")
                for m in range(2):
                    nc.tensor.matmul(cps[:, m * 5:(m + 1) * 5],
                                     ept[:, m * P:(m + 1) * P], smat,
                                     start=True, stop=True)
                # copy out of PSUM right away: the main matmuls reuse this bank
                cpss = prep.tile([P, 10], f32, tag="cpss")
                nc.vector.tensor_scalar_add(cpss[:], cps[:], 0.0)
                cpsv = cpss[:].rearrange("p (m c) -> p c m", m=2)

                # exp -> fp8 DoubleRow weights + natural-layout f32 for sums
                nc.scalar.activation(out=e8T[:, 0], in_=wt1[:, 0], func=Act.Exp)
                nc.scalar.activation(out=e_nat[:],
                                     in_=wt2[:, 0:1024].rearrange("p (w m c) -> p w m c", w=2, m=2),
                                     func=Act.Exp)
                nc.scalar.activation(out=e8T[:, 1], in_=wt1[:, 1], func=Act.Exp)

                # row sums of exp(w - shift): [p, which, m]
                rs4 = prep.tile([P, 2, 2], f32, tag="rs4")
                for wsel in range(2):
                    for m in range(2):
                        nc.vector.tensor_reduce(out=rs4[:, wsel, m:m + 1],
                                                in_=e_nat[:, wsel, m, :],
                                                axis=mybir.AxisListType.X, op=Alu.add)

                sa2 = rs4[:, 0, :]
                sb2 = rs4[:, 1, :]
                rpts = prep.tile([P, 2], f32, tag="rpts")
                nc.vector.reciprocal(out=rpts[:], in_=cpsv[:, 0, :])
                rcab = prep.tile([P, 2], f32, tag="rcab")
                nc.vector.reciprocal(out=rcab[:], in_=cpsv[:, 4, :])
                rsa = prep.tile([P, 2], f32, tag="rsa")
                nc.vector.reciprocal(out=rsa[:], in_=sa2)
                rsb = prep.tile([P, 2], f32, tag="rsb")
                nc.vector.reciprocal(out=rsb[:], in_=sb2)

                h = prep.tile([P, 2], f32, tag="h")
                nc.vector.tensor_tensor(out=h[:], in0=rpts[:], in1=rsa[:], op=Alu.mult)
                nc.vector.tensor_tensor(out=bi2[:], in0=cpsv[:, 2, :], in1=h[:], op=Alu.mult)
                h2 = prep.tile([P, 2], f32, tag="h2")
                nc.vector.tensor_tensor(out=h2[:], in0=h[:], in1=rsb[:], op=Alu.mult)
                nc.vector.tensor_tensor(out=sc2[:], in0=cpsv[:, 4, :], in1=h2[:], op=Alu.mult)
                g = prep.tile([P, 2], f32, tag="g")
                nc.vector.tensor_tensor(out=g[:], in0=cpsv[:, 3, :], in1=rcab[:], op=Alu.mult)
                nc.vector.tensor_tensor(out=U2[:], in0=g[:], in1=sa2, op=Alu.mult)
                t = prep.tile([P, 2], f32, tag="t")
                nc.vector.tensor_tensor(out=t[:], in0=cpsv[:, 2, :], in1=g[:], op=Alu.mult)
                t2 = prep.tile([P, 2], f32, tag="t2")
                nc.vector.tensor_tensor(out=t2[:], in0=cpsv[:, 1, :], in1=t[:], op=Alu.subtract)
                nc.vector.tensor_tensor(out=w2[:], in0=t2[:], in1=rpts[:], op=Alu.mult)


            # ---- main loop ----
            with tc.tile_pool(name="ep", bufs=3) as ep, \
                 tc.tile_pool(name="mm_ps", bufs=1, space="PSUM") as mm_ps:
                for n in range(NCH):
                    xk = xtiles[n]
                    sl_n = slice(n * CH, (n + 1) * CH)
                    for m in range(2):
                        a_ps = mm_ps.tile([P, CH], f32, tag=f"A{m}", name=f"A{n}{m}")
                        b_ps = mm_ps.tile([P, CH], f32, tag=f"B{m}", name=f"B{n}{m}")
                        for ps_t, wsel in ((a_ps, 0), (b_ps, 1)):
                            wT = e8T[:, wsel, :, m * P:(m + 1) * P]
                            for s in range(CH // 512):
                                sl = slice(s * 512, (s + 1) * 512)
                                nc.tensor.matmul(ps_t[:, sl], wT, xk[:, :, sl],
                                                 start=True, stop=True, perf_mode=DR)
                        # out = (A + U) * (sc*B + bi) + w
                        s_sb = ep.tile([P, CH], f32, tag="s", name=f"s{n}{m}")
                        nc.scalar.activation(out=s_sb[:], in_=b_ps[:], func=Act.Identity,
                                             scale=sc2[:, m:m + 1], bias=bi2[:, m:m + 1])
                        p_sb = ep.tile([P, CH], f32, tag="p", name=f"p{n}{m}")
                        nc.vector.scalar_tensor_tensor(
                            out=p_sb[:], in0=a_ps[:], scalar=U2[:, m:m + 1],
                            in1=s_sb[:], op0=Alu.add, op1=Alu.mult)
                        if m == 0:
                            o_sb = ep.tile([P, CH], bf, tag="ob", name=f"o{n}{m}")
                            nc.scalar.activation(out=o_sb[:], in_=p_sb[:],
                                                 func=Act.Identity, bias=w2[:, m:m + 1])
                            nc.sync.dma_start(out=outb_d[:, sl_n], in_=o_sb[:])
                        else:
                            o_sb = ep.tile([P, CH], f32, tag="of", name=f"o{n}{m}")
                            nc.vector.tensor_scalar_add(o_sb[:], p_sb[:], w2[:, m:m + 1])
                            nc.sync.dma_start(out=outf_d[:, sl_n], in_=o_sb[:])

    nc.compile()
    return nc


def _get_nc():
    if "nc" not in _CACHE:
        _CACHE["nc"] = _build_bass()
    return _CACHE["nc"]


def _prep_weights(wa, wb, wt):
    """Host-side layout/dtype prep: stability shift + transpose of the exp
    inputs (softmax itself — exp, sums, normalization — runs on device)."""
    lg = math.log(128.0)
    sha = wa.max(axis=1, keepdims=True) - lg
    shb = wb.max(axis=1, keepdims=True) - lg

    def tr(w, sh):  # [size, prev] -> [p, kblock, size]
        return np.ascontiguousarray(
            (w - sh).T.reshape(2, P, SIZE).transpose(1, 0, 2))

    def nat(w, sh):  # [size, prev] -> [p, (m, c)]
        return (w - sh).reshape(2, P, PREV).transpose(1, 0, 2).reshape(P, 2 * PREV)

    wt1 = np.stack([tr(wa, sha), tr(wb, shb)], axis=1)  # [p, which, k, size]

    wt2 = np.zeros((P, WT2_W), dtype=np.float32)
    wt2[:, 0:512] = nat(wa, sha)
    wt2[:, 512:1024] = nat(wb, shb)
    wt2[:16, 1024:1029] = _sign_matrix()
    wt2[:16, 1029:1285] = wt
    return np.ascontiguousarray(wt1), wt2


def _run(x, wa, wb, wt, trace=False, **spmd_kwargs):
    from concourse import bass_utils

    nc = _get_nc()
    x = np.asarray(x, dtype=np.float32)
    wa = np.asarray(wa, dtype=np.float32)
    wb = np.asarray(wb, dtype=np.float32)
    wt = np.asarray(wt, dtype=np.float32)
    wt1, wt2 = _prep_weights(wa, wb, wt)

    # [256, batch] f32 -> [p, kblock, batch] fp8
    x8 = np.ascontiguousarray(
        x.astype(FP8).reshape(2, P, BATCH).transpose(1, 0, 2))

    in_maps = []
    for c in range(N_CORES):
        in_maps.append({
            "xs": np.ascontiguousarray(x8[:, :, c * BSH:(c + 1) * BSH]),
            "wt1": wt1, "wt2": wt2,
        })
    res = bass_utils.run_bass_kernel_spmd(nc, in_maps, core_ids=list(range(N_CORES)),
                                          trace=trace, **spmd_kwargs)
    outs = []
    for c in range(N_CORES):
        full = np.empty((SIZE, BSH), dtype=np.float32)
        full[0:P] = np.asarray(res.results[c]["outb"]).astype(np.float32)
        full[P:SIZE] = np.asarray(res.results[c]["outf"])
        outs.append(full)
    return np.concatenate(outs, axis=1), res


def kernel(x, wa, wb, wt):
    out, _ = _run(x, wa, wb, wt, trace=False)
    return out
